# revision 10
# baseline (speedup 1.0000x reference)
"""Causal self-attention + residual + LayerNorm fused Trainium2 kernel.

Problem: B=4, S=2048, D=1024, H=16 heads (hd=64), fp32 in/out.
    qkv = x @ in_proj_w.T + in_proj_b ; causal MHA ; out proj ; y = LN(x + attn_out)

Sharding (zero cross-core communication, 8 NeuronCores):
    core c -> batch b = c % 4, query-group g = c // 4.
    Causal zig-zag balance: g=0 owns query blocks [0:512) and [1536:2048),
    g=1 owns [512:1536). Every core computes full K/V for its batch
    (keys 0:2048), attention only for its own queries, then out-proj +
    residual + LayerNorm for its queries. Outputs are disjoint row sets.

Layout: everything is computed transposed (features on partitions,
tokens on the free axis), which makes every matmul contraction land on
the partition axis with zero on-chip transposes:
    K^T[f,t] / Q^T[f,q] = W^T-tile.T @ x^T        (lhsT = in_proj_w.T tile)
    V[t,f]              = x^T-tile.T @ W^T        (lhsT = x^T tile)
    S^T[k,q]            = K^T-slice.T @ Q^T-slice (contraction = head dim 64,
                                                   two heads packed in the PE
                                                   array via tile_position)
    ctx^T[d,q]          = V-slice.T @ exp(S^T)    (V augmented with a ones
                                                   column -> row 64 of the
                                                   PSUM tile = softmax denom)
    out^T[Do,q]         = out_w.T-tile.T @ ctx^T
    LN stats            = ones.T @ y / ones.T @ y^2 (partition reduction on PE)
Matmuls run in float32r (TF32-like, ~11-bit mantissa, 4x faster than fp32
on the PE; measured end-to-end relerr ~1.5e-4). Softmax skips the max
subtraction (scores ~ N(0,1)) and defers the divide: ctx is normalized by
the reciprocal of the aug-row denominator, broadcast across partitions by
the GPSIMD partition_broadcast op.

The two query-groups differ only in the per-q-tile causal k-tile counts;
both variants are emitted under a tc.If on the partition id, so one SPMD
program serves all 8 cores in a single launch.
"""
import sys

if "/opt/trn_rl_repo" not in sys.path:
    sys.path.insert(0, "/opt/trn_rl_repo")

import numpy as np

B, S, D, H, HD = 4, 2048, 1024, 16, 64
P = 128
QT = 512                      # queries per q-tile (matmul free dim)
NQ = 1024                     # queries per core
NKT = S // P                  # 16 k-tiles per batch
DK = D // P                   # 8 contraction tiles over D
NPLAIN = {0: (0, 12), 1: (4, 8)}   # group -> per-q-tile plain (unmasked) k-tiles

_cache = {}


def _build():
    import concourse.mybir as mybir
    import concourse.tile as tile
    from concourse import bacc
    from concourse.bass import ts
    from concourse.alu_op_type import AluOpType

    f32 = mybir.dt.float32
    f32r = mybir.dt.float32r
    AF = mybir.ActivationFunctionType

    nc = bacc.Bacc("TRN2", target_bir_lowering=False, debug=False, num_devices=8)

    xkv = nc.dram_tensor("xkv", [D, S], f32r, kind="ExternalInput").ap()
    xq = nc.dram_tensor("xq", [D, NQ], f32r, kind="ExternalInput").ap()
    wt = nc.dram_tensor("wt", [D, 3 * D], f32r, kind="ExternalInput").ap()
    wot = nc.dram_tensor("wot", [D, D], f32r, kind="ExternalInput").ap()
    maskd = nc.dram_tensor("maskd", [P, 896], f32r, kind="ExternalInput").ap()
    bqd = nc.dram_tensor("bqd", [D], f32, kind="ExternalInput").ap()
    bkd = nc.dram_tensor("bkd", [D], f32, kind="ExternalInput").ap()
    bvd = nc.dram_tensor("bvd", [D], f32, kind="ExternalInput").ap()
    bod = nc.dram_tensor("bod", [D], f32, kind="ExternalInput").ap()
    gamd = nc.dram_tensor("gamd", [D], f32, kind="ExternalInput").ap()
    betd = nc.dram_tensor("betd", [D], f32, kind="ExternalInput").ap()
    yt = nc.dram_tensor("yt", [D, NQ], f32, kind="ExternalOutput").ap()

    xkv_r = xkv.rearrange("(dk p) t -> p dk t", p=P)
    xq_r = xq.rearrange("(dk p) q -> p dk q", p=P)
    xq_f32 = xq.bitcast(f32).rearrange("(ok p) q -> p ok q", p=P)

    with tile.TileContext(nc) as tc:
        with (
            tc.tile_pool(name="persist", bufs=1) as pers,
            tc.tile_pool(name="proj_ps", bufs=2, space="PSUM") as proj_ps,
        ):
            kt = pers.tile([P, DK, S], f32r)           # K^T       64 KB/part
            msk = pers.tile([P, 896], f32r)            #           3.5 KB
            bia = pers.tile([P, DK, 6], f32)           # bq bk bv bo gam bet
            ones128 = pers.tile([P, 1], f32r)
            eps_t = pers.tile([1, 1], f32)
            nc.vector.memset(eps_t[:], 1e-5)

            nc.sync.dma_start(msk[:], maskd[:])
            for j, src in enumerate((bqd, bkd, bvd, bod, gamd, betd)):
                nc.sync.dma_start(bia[:, :, j], src.rearrange("(f p) -> p f", p=P))
            nc.vector.memset(ones128[:].bitcast(f32), 1.0)

            def bq_(f): return bia[:, f, 0:1]
            def bk_(f): return bia[:, f, 1:2]
            def bo_(f): return bia[:, f, 3:4]
            def gam_(f): return bia[:, f, 4:5]
            def bet_(f): return bia[:, f, 5:6]

            # ---- phase A: K^T projection ------------------------------
            with (
                tc.tile_pool(name="wk", bufs=1) as wkp,
                tc.tile_pool(name="xa", bufs=2) as xap,
            ):
                wk = wkp.tile([P, DK, DK, P], f32r)
                nc.sync.dma_start(
                    wk[:],
                    wt[:, D:2 * D].rearrange("(dk p) (f c) -> p dk f c", p=P, c=P),
                )
                for t in range(S // QT):
                    xc = xap.tile([P, DK, QT], f32r, tag="xa")
                    nc.sync.dma_start(xc[:], xkv_r[:, :, ts(t, QT)])
                    for f in range(DK):
                        ps = proj_ps.tile([P, QT], f32, tag="pp")
                        for dk in range(DK):
                            nc.tensor.matmul(
                                ps[:], wk[:, dk, f, :], xc[:, dk, :],
                                start=(dk == 0), stop=(dk == DK - 1),
                            )
                        nc.vector.tensor_scalar_add(kt[:, f, ts(t, QT)], ps[:], bk_(f))

            with tc.tile_pool(name="vpool", bufs=1) as vp:
                v = vp.tile([P, NKT, H, HD + 1], f32r)   # V aug  65 KB/part
                nc.vector.memset(v[:, :, :, HD].bitcast(f32), 1.0)

                # ---- phase B: V projection (natural orientation) ------
                with (
                    tc.tile_pool(name="wv", bufs=1) as wvp,
                    tc.tile_pool(name="xb", bufs=4) as xbp,
                ):
                    wv = wvp.tile([P, DK, 2, 512], f32r)
                    nc.sync.dma_start(
                        wv[:],
                        wt[:, 2 * D:3 * D].rearrange(
                            "(dk p) (g c) -> p dk g c", p=P, c=512),
                    )
                    for t in range(NKT):
                        xc = xbp.tile([P, DK, P], f32r, tag="xb")
                        nc.sync.dma_start(xc[:], xkv_r[:, :, ts(t, P)])
                        for fg in range(2):
                            ps = proj_ps.tile([P, 512], f32, tag="pp")
                            for dk in range(DK):
                                nc.tensor.matmul(
                                    ps[:], xc[:, dk, :], wv[:, dk, fg, :],
                                    start=(dk == 0), stop=(dk == DK - 1),
                                )
                            for hh in range(8):
                                h = 8 * fg + hh
                                nc.vector.tensor_copy(
                                    v[:, t, h, 0:HD], ps[:, ts(hh, HD)]
                                )

                # ---- phases C-F under the partition-id branch ---------
                with tc.tile_pool(name="qc", bufs=1) as qcp:
                    ctx = qcp.tile([P, DK, QT], f32r)

                    def qproj(qt, qtile):
                        with (
                            tc.tile_pool(name="wq", bufs=2) as wqp,
                            tc.tile_pool(name="xqp", bufs=1) as xqp,
                        ):
                            xc = xqp.tile([P, DK, QT], f32r, tag="xq")
                            nc.sync.dma_start(xc[:], xq_r[:, :, ts(qt, QT)])
                            for f in range(DK):
                                wq = wqp.tile([P, DK, P], f32r, tag="wq")
                                nc.sync.dma_start(
                                    wq[:],
                                    wt[:, ts(f, P)].rearrange(
                                        "(dk p) c -> p dk c", p=P),
                                )
                                ps = proj_ps.tile([P, QT], f32, tag="pp")
                                for dk in range(DK):
                                    nc.tensor.matmul(
                                        ps[:], wq[:, dk, :], xc[:, dk, :],
                                        start=(dk == 0), stop=(dk == DK - 1),
                                    )
                                nc.vector.tensor_scalar_add(
                                    qtile[:, f, :], ps[:], bq_(f))

                    def attn(n_plain, qtile):
                        nk = n_plain + 4
                        with (
                            tc.tile_pool(name="sep", bufs=4) as sep,
                            tc.tile_pool(name="scr", bufs=2) as scr,
                            tc.tile_pool(name="s_ps", bufs=2, space="PSUM") as s_ps,
                            tc.tile_pool(name="c_ps", bufs=2, space="PSUM") as c_ps,
                        ):
                            for hp in range(H // 2):
                                cp0 = c_ps.tile([HD + 1, QT], f32, tag="c0")
                                cp1 = c_ps.tile([HD + 1, QT], f32, tag="c1")
                                for i in range(nk):
                                    sp0 = s_ps.tile([P, QT], f32, tag="s")
                                    sp1 = s_ps.tile([P, QT], f32, tag="s")
                                    nc.tensor.matmul(
                                        sp0[:], kt[0:HD, hp, ts(i, P)],
                                        qtile[0:HD, hp, :], start=True, stop=True,
                                    )
                                    nc.tensor.matmul(
                                        sp1[:], kt[HD:P, hp, ts(i, P)],
                                        qtile[HD:P, hp, :], start=True, stop=True,
                                    )
                                    se0 = sep.tile([P, QT], f32r, tag="se")
                                    se1 = sep.tile([P, QT], f32r, tag="se")
                                    nc.scalar.activation(
                                        se0[:], sp0[:], AF.Exp, scale=0.125)
                                    nc.scalar.activation(
                                        se1[:], sp1[:], AF.Exp, scale=0.125)
                                    if i >= n_plain:
                                        off = 384 - P * (i - n_plain)
                                        nc.vector.tensor_mul(
                                            se0[:], se0[:], msk[:, off:off + QT])
                                        nc.vector.tensor_mul(
                                            se1[:], se1[:], msk[:, off:off + QT])
                                    nc.tensor.matmul(
                                        cp0[:], v[:, i, 2 * hp, :], se0[:],
                                        start=(i == 0), stop=(i == nk - 1),
                                    )
                                    nc.tensor.matmul(
                                        cp1[:], v[:, i, 2 * hp + 1, :], se1[:],
                                        start=(i == 0), stop=(i == nk - 1),
                                    )
                                for j, cp in ((0, cp0), (1, cp1)):
                                    h = 2 * hp + j
                                    po, ft = HD * (h % 2), h // 2
                                    rec = scr.tile([1, QT], f32, tag="rec")
                                    nc.vector.reciprocal(rec[:], cp[HD:HD + 1, :])
                                    bc = scr.tile([HD, QT], f32, tag="bc")
                                    nc.gpsimd.partition_broadcast(bc[:], rec[:])
                                    dst = ctx[po:po + HD, ft, :]
                                    nc.vector.tensor_mul(dst, cp[0:HD, :], bc[:])
                                    nc.vector.tensor_scalar_add(
                                        dst, dst, bia[po:po + HD, ft, 2:3])

                    def outproj_ln(qt):
                        with (
                            tc.tile_pool(name="wo", bufs=3) as wop,
                            tc.tile_pool(name="ep", bufs=1) as ep,
                            tc.tile_pool(name="st_ps", bufs=2, space="PSUM") as st_ps,
                        ):
                            y = ep.tile([P, DK, QT], f32r, tag="y")
                            for o in range(DK):
                                wo = wop.tile([P, DK, P], f32r, tag="wo")
                                nc.sync.dma_start(
                                    wo[:],
                                    wot[:, ts(o, P)].rearrange(
                                        "(dk p) c -> p dk c", p=P),
                                )
                                ps = proj_ps.tile([P, QT], f32, tag="pp")
                                for dk in range(DK):
                                    nc.tensor.matmul(
                                        ps[:], wo[:, dk, :], ctx[:, dk, :],
                                        start=(dk == 0), stop=(dk == DK - 1),
                                    )
                                xr = ep.tile([P, QT], f32, tag="xr", bufs=2)
                                nc.sync.dma_start(xr[:], xq_f32[:, o, ts(qt, QT)])
                                nc.vector.scalar_tensor_tensor(
                                    y[:, o, :], ps[:], bo_(o), xr[:],
                                    AluOpType.add, AluOpType.add,
                                )
                            mu_ps = st_ps.tile([1, QT], f32, tag="mu")
                            for o in range(DK):
                                nc.tensor.matmul(
                                    mu_ps[:], ones128[:], y[:, o, :],
                                    start=(o == 0), stop=(o == DK - 1))
                            ms_ps = st_ps.tile([1, QT], f32, tag="ms")
                            for o in range(DK):
                                ysq = ep.tile([P, QT], f32r, tag="ysq")
                                nc.vector.tensor_mul(
                                    ysq[:], y[:, o, :], y[:, o, :])
                                nc.tensor.matmul(
                                    ms_ps[:], ones128[:], ysq[:],
                                    start=(o == 0), stop=(o == DK - 1))
                            mu = ep.tile([1, QT], f32, tag="mu_sb")
                            nc.scalar.mul(mu[:], mu_ps[:], 1.0 / D)
                            ms = ep.tile([1, QT], f32, tag="ms_sb")
                            nc.scalar.mul(ms[:], ms_ps[:], 1.0 / D)
                            tmp = ep.tile([1, QT], f32, tag="stat_tmp", bufs=2)
                            nc.vector.tensor_mul(tmp[:], mu[:], mu[:])
                            nc.vector.tensor_sub(ms[:], ms[:], tmp[:])  # var
                            sd = ep.tile([1, QT], f32, tag="stat_tmp", bufs=2)
                            nc.scalar.activation(sd[:], ms[:], AF.Sqrt, bias=eps_t[:])
                            rstd = ep.tile([1, QT], f32, tag="rstd")
                            nc.vector.reciprocal(rstd[:], sd[:])
                            mu_bc = ep.tile([P, QT], f32, tag="mu_bc")
                            nc.gpsimd.partition_broadcast(mu_bc[:], mu[:])
                            rs_bc = ep.tile([P, QT], f32, tag="rs_bc")
                            nc.gpsimd.partition_broadcast(rs_bc[:], rstd[:])
                            for o in range(DK):
                                t1 = ep.tile([P, QT], f32, tag="t1", bufs=2)
                                nc.vector.tensor_sub(
                                    t1[:], y[:, o, :].bitcast(f32), mu_bc[:])
                                nc.vector.tensor_mul(t1[:], t1[:], rs_bc[:])
                                yo = ep.tile([P, QT], f32, tag="yo", bufs=2)
                                nc.vector.tensor_scalar(
                                    yo[:], t1[:], gam_(o), bet_(o),
                                    AluOpType.mult, AluOpType.add,
                                )
                                nc.sync.dma_start(yt[ts(o, P), ts(qt, QT)], yo[:])

                    def group(g):
                        for qt in range(2):
                            with tc.tile_pool(name="qtp", bufs=1) as qtp:
                                qtile = qtp.tile([P, DK, QT], f32r, tag="qtile")
                                qproj(qt, qtile)
                                attn(NPLAIN[g][qt], qtile)
                            outproj_ln(qt)

                    pid = nc.partition_id()
                    with tc.If(pid < 4) as cmp:
                        group(0)
                    with cmp.Else():
                        group(1)
    nc.compile()
    return nc


def _get_nc():
    if "nc" not in _cache:
        _cache["nc"] = _build()
    return _cache["nc"]


def _prep(x, in_proj_w, in_proj_b, out_w, out_b, gamma, beta):
    x = np.asarray(x, np.float32)
    wt = np.ascontiguousarray(np.asarray(in_proj_w, np.float32).T)
    wot = np.ascontiguousarray(np.asarray(out_w, np.float32).T)
    bqkv = np.asarray(in_proj_b, np.float32)
    bo = np.asarray(out_b, np.float32)
    gam = np.asarray(gamma, np.float32)
    bet = np.asarray(beta, np.float32)
    ku = np.arange(P)[:, None] <= (np.arange(896)[None, :] - 384)
    maskd = ku.astype(np.float32)
    qcols = {
        0: np.r_[0:QT, 3 * QT:4 * QT],
        1: np.r_[QT:3 * QT],
    }
    in_maps = []
    for c in range(8):
        b, g = c % 4, c // 4
        xt = np.ascontiguousarray(x[b].T)
        in_maps.append({
            "xkv": xt,
            "xq": np.ascontiguousarray(xt[:, qcols[g]]),
            "wt": wt,
            "wot": wot,
            "maskd": maskd,
            "bqd": bqkv[0:D], "bkd": bqkv[D:2 * D], "bvd": bqkv[2 * D:3 * D],
            "bod": bo, "gamd": gam, "betd": bet,
        })
    return in_maps, qcols


def _run(in_maps, trace=False, **kw):
    from concourse.bass_utils import run_bass_kernel_spmd

    return run_bass_kernel_spmd(_get_nc(), in_maps, list(range(8)), trace=trace, **kw)


def kernel(x, in_proj_w, in_proj_b, out_w, out_b, gamma, beta):
    in_maps, qcols = _prep(x, in_proj_w, in_proj_b, out_w, out_b, gamma, beta)
    res = _run(in_maps)
    out = np.empty((B, S, D), np.float32)
    for c in range(8):
        out[c % 4, qcols[c // 4]] = res.results[c]["yt"].T
    return out
